# revision 1
# baseline (speedup 1.0000x reference)
"""Trainium2 Bass kernel for nn_CAPMemory (camera-aware proxy memory loss).

Strategy (8 NeuronCores, SPMD, no collectives):
  - Shard the 64000x256 proxy/center table over P: core k owns centers rows
    [8000k, 8000(k+1)) = 1000 labels x 8 cams (contiguous), transposed on the
    host to [256, 8000] for direct use as the matmul moving operand (float32r
    runs the PE at 1 cycle/row vs 4 for float32).
  - Batch rows (512) are replicated on every core, permuted so camera groups
    are contiguous and big/small-paired so most 128-row tiles span only ~2
    cameras; the intra-loss camera selection is then a handful of full-width
    strided-AP exp+accum instructions.
  - Each core computes its [512, 8000] slice of the (unnormalized) similarity
    matrix on the PE (raw feats transposed on device; the 1/||f|| scale rides
    in the intra exp scale and in host post-processing, which keeps the
    normalization off the critical path), then reduces it on device to small
    per-core outputs:
      cand [512, 8*8]  top-8 values of each ~1024-column chunk (DVE InstMax)
      srow [512, 8]    sum_l exp(20/||f|| * rawsims) per camera slot present
                       in the row-tile (ACT Exp + accumulate, no max-shift:
                       20*cos() in [-20, 20] is comfortably inside f32)
  - Schedule: column-group 0 for all row-tiles first (PE dense during the DMA
    fill), then row-tile-major so each tile's exps overlap later compute;
    PSUM->SBUF copies mostly on ACT, a few on DVE, to balance busy time.
  - Host merge: intra logsumexp = log(sum_k srow_k); global top-50 hard
    negatives from the 8x64 candidates with the label-masked (positive)
    columns removed by eps-value-matching; positives (8 values/row, 0.01% of
    the similarity matrix) are computed on host in f64.
  - Exactness certificate: every chunk's 8th-largest value must be <= the
    50th-largest merged candidate; rows violating it (empirically zero; the
    probability is ~1e-4 per run on random data) are recomputed exactly on
    host from the raw inputs, so the result stays correct regardless.
"""

import sys
import functools

sys.path.insert(0, "/opt/trn_rl_repo")

import numpy as np

from concourse import bacc, mybir
from concourse.tile import TileContext

F32 = mybir.dt.float32

N = 512          # batch
D = 256          # feature dim
L = 8000         # labels
C = 8            # cameras
P_LOCAL = 8000   # center columns per core (= 1000 labels * 8 cams)
L_LOCAL = 1000   # labels per core
NCORES = 8
RT = 4           # row tiles of 128
INV_T = 20.0     # 1 / temperature
K = 50           # hard negatives
LW = 0.5         # inter-cam loss weight

# matmul column chunks: 512-wide (one full PSUM bank, contiguous copies,
# multiples of 8 so the camera stride survives); tail chunk 320
MM_STARTS = [(j * 512, min(512, P_LOCAL - j * 512)) for j in range(16)]
# PSUM tile groups: 4 banks each -> column spans
PS_GROUPS = [(0, 2048), (2048, 2048), (4096, 2048), (6144, 1856)]
N_PS = len(PS_GROUPS)
# top-8 extraction chunks (starts, sizes); 8-aligned boundaries
MAX_STARTS = [(i * 1024, min(1024, P_LOCAL - i * 1024)) for i in range(8)]
N_MAXCH = len(MAX_STARTS)          # 8
CAND = N_MAXCH * 8                 # 64 candidate values per row per core

# matmul operand dtype: float32 (exact, PE 4 cyc/row), float32r (PE 1 cyc/row
# at moving dim >= 256), bfloat16 (1 cyc/row + half DMA)
MM_DT = mybir.dt.float32r
MM_NP = np.float32


def _pair_order(sizes):
    """Order cameras big+small so most 128-row tiles span only 2 cameras."""
    desc = np.argsort(-np.asarray(sizes), kind="stable")
    big, small = desc[: C // 2], desc[C // 2 :][::-1]
    order = []
    for b, s in zip(big, small):
        order += [int(b), int(s)]
    return order


@functools.lru_cache(maxsize=8)
def _build_program(tile_cams, repeats=1):
    nc = bacc.Bacc(None, target_bir_lowering=False, num_swdge_queues=4)

    cenT = nc.dram_tensor("cenT", [2, 128, P_LOCAL], MM_DT, kind="ExternalInput")
    featsd = nc.dram_tensor("feats", [RT, 128, D], F32, kind="ExternalInput")
    identd = nc.dram_tensor("ident", [128, 128], F32, kind="ExternalInput")
    candd = nc.dram_tensor("cand", [RT, 128, CAND], F32, kind="ExternalOutput")
    srowd = nc.dram_tensor("srow", [RT, 128, C], F32, kind="ExternalOutput")

    with TileContext(nc) as tc:
        with (
            tc.tile_pool(name="cen", bufs=1) as cenp,
            tc.tile_pool(name="ftp", bufs=1) as ftp,
            tc.tile_pool(name="simsp", bufs=2) as simsp,
            tc.tile_pool(name="smallp", bufs=2) as smallp,
            tc.tile_pool(name="outp", bufs=2) as outp,
            tc.tile_pool(name="psum", bufs=2, space="PSUM") as psump,
        ):
            for _rep in range(repeats):
                _kernel_body(nc, tc, cenp, ftp, simsp, smallp, outp, psump,
                             cenT, featsd, identd, candd, srowd, tile_cams)

    nc.compile()
    return nc


def _kernel_body(nc, tc, cenp, ftp, simsp, smallp, outp, psump,
                 cenT, featsd, identd, candd, srowd, tile_cams):
    ActF = mybir.ActivationFunctionType
    Axis = mybir.AxisListType

    # small transfers first so the feats pipeline starts immediately
    ident_sb = smallp.tile([128, 128], F32, name="ident_sb", bufs=1)
    nc.sync.dma_start(out=ident_sb[:, :], in_=identd[:, :])
    # preload the Exp LUT in ACT's only idle window (before feats arrive)
    warm = smallp.tile([128, 1], F32, name="warm", bufs=1)
    nc.scalar.activation(warm[:, 0:1], ident_sb[:, 0:1], ActF.Exp)
    ftiles = []
    for rt in range(RT):
        ftile = smallp.tile([128, D], F32, name="ftile", bufs=4)
        nc.sync.dma_start(out=ftile[:, :], in_=featsd[rt])
        ftiles.append(ftile)

    # centers: interleave (chunk, k-half) so the first matmuls unblock early,
    # and alternate issuing engines to spread the transfers across queues
    cen_sb = [
        cenp.tile([128, P_LOCAL], MM_DT, name="cen0"),
        cenp.tile([128, P_LOCAL], MM_DT, name="cen1"),
    ]
    dma_engines = [nc.sync, nc.gpsimd]
    for j in range(8):
        s = slice(j * 1000, (j + 1) * 1000)
        for kh in range(2):
            eng = dma_engines[(2 * j + kh) % len(dma_engines)]
            eng.dma_start(out=cen_sb[kh][:, s], in_=cenT[kh, :, s])

    # transpose RAW feats for the matmul; the 1/||f|| normalization is folded
    # into the PSUM->SBUF copy as a per-partition scale, off the critical path
    fTs = []
    for rt in range(RT):
        fT0 = ftp.tile([128, 128], MM_DT, name=f"fT{rt}_0")
        fT1 = ftp.tile([128, 128], MM_DT, name=f"fT{rt}_1")
        for kh, fT in ((0, fT0), (1, fT1)):
            pt = psump.tile([128, 4, 512], F32, name="ps")
            nc.tensor.transpose(
                pt[:, 0, 0:128], ftiles[rt][:, kh * 128 : (kh + 1) * 128],
                ident_sb[:, :]
            )
            if rt % 2 == 1:
                nc.vector.tensor_copy(fT[:, :], pt[:, 0, 0:128])
            else:
                nc.scalar.copy(fT[:, :], pt[:, 0, 0:128])
        fTs.append((fT0, fT1))

    # row norms: squares/reductions on DVE, one batched sqrt + reciprocal
    n2 = smallp.tile([128, RT], F32, name="n2", bufs=1)
    nrm = smallp.tile([128, RT], F32, name="nrm", bufs=1)
    inv = smallp.tile([128, RT], F32, name="inv", bufs=1)
    for rt in range(RT):
        fsq = smallp.tile([128, D], F32, name="fsq")
        nc.vector.tensor_mul(fsq[:, :], ftiles[rt][:, :], ftiles[rt][:, :])
        nc.vector.reduce_sum(n2[:, rt : rt + 1], fsq[:, :], axis=Axis.X)
    nc.scalar.sqrt(nrm[:, :], n2[:, :])
    nc.vector.reciprocal(inv[:, :], nrm[:, :])
    sc20 = smallp.tile([128, RT], F32, name="sc20", bufs=1)
    nc.vector.tensor_scalar_mul(sc20[:, :], inv[:, :], INV_T)

    # schedule: group 0 for all row-tiles first (keeps PE dense while the
    # center DMA stream fills), then row-tile-major so each tile's intra exps
    # overlap later tiles' compute
    sims_t = [
        simsp.tile([128, P_LOCAL], F32, name=f"sims{rt}", bufs=1)
        for rt in range(RT)
    ]
    cand_ts = [
        outp.tile([128, CAND], F32, name=f"cand{rt}", bufs=1) for rt in range(RT)
    ]
    schedule = [(0, rt) for rt in range(RT)] + [
        (pk, rt) for rt in range(RT) for pk in range(1, N_PS)
    ]
    for pk, rt in schedule:
        if True:
            g0, glen = PS_GROUPS[pk]
            sims = sims_t[rt]
            ps = psump.tile([128, 4, 512], F32, name="ps")
            nmm = (glen + 511) // 512
            for mk in range(nmm):
                lo = g0 + mk * 512
                w = min(512, g0 + glen - lo)
                s = slice(lo, lo + w)
                nc.tensor.matmul(
                    ps[:, mk, 0:w], fTs[rt][0][:, :], cen_sb[0][:, s],
                    start=True, stop=False,
                )
                nc.tensor.matmul(
                    ps[:, mk, 0:w], fTs[rt][1][:, :], cen_sb[1][:, s],
                    start=False, stop=True,
                )
            # plain PSUM->SBUF copies (contiguous); sims stay UNNORMALIZED on
            # device (top-8 order is unchanged; host + exp scale apply 1/||f||)
            # 2 of 16 copies on DVE to balance ACT/DVE busy time
            eng_copy = (
                nc.vector.tensor_copy if (pk == 1 and rt in (1, 3))
                else nc.scalar.copy
            )
            if glen == 2048:
                eng_copy(sims[:, g0 : g0 + 2048], ps[:, :, :])
            else:
                eng_copy(sims[:, g0 : g0 + 1536], ps[:, 0:3, :])
                eng_copy(sims[:, g0 + 1536 : g0 + glen], ps[:, 3, 0 : glen - 1536])
            for h in (2 * pk, 2 * pk + 1):
                j0, hlen = MAX_STARTS[h]
                nc.vector.max(
                    cand_ts[rt][:, h * 8 : h * 8 + 8], sims[:, j0 : j0 + hlen]
                )

            if pk == N_PS - 1:
                # intra: one full-width strided exp+accum per camera present
                # in this row-tile (no max-shift; 20*x in [-20, 20] fits f32).
                # Each camera writes its own slot of s_t; the host picks each
                # row's slot from the camera order. Rows with a different
                # camera compute garbage in that slot; the host ignores it.
                scr = smallp.tile(
                    [128, L_LOCAL], mybir.dt.bfloat16, name="scr", bufs=1
                )
                s_t = smallp.tile([128, C], F32, name="s_t")
                simsr = sims.rearrange("p (l c) -> p l c", c=C)
                for idx, cam in enumerate(tile_cams[rt]):
                    nc.scalar.activation(
                        scr[:, :], simsr[:, :, cam], ActF.Exp,
                        scale=sc20[:, rt : rt + 1],
                        accum_out=s_t[:, idx : idx + 1],
                    )
                nc.gpsimd.dma_start(out=candd[rt], in_=cand_ts[rt][:, :])
                nc.gpsimd.dma_start(out=srowd[rt], in_=s_t[:, :])



class _Runner:
    """Sharded 8-core executor for a built Bass program.

    Builds the jax.jit(shard_map(bass_exec)) executable once (the walrus/NEFF
    compile happens inside the first call) and reuses it for every subsequent
    execution, keeping large inputs device-resident.
    """

    def __init__(self, nc, n_cores=NCORES):
        import jax
        from jax.sharding import Mesh, PartitionSpec, NamedSharding
        from jax.experimental.shard_map import shard_map
        from concourse import bass2jax

        self.jax = jax
        self.nc = nc
        self.n_cores = n_cores
        bass2jax.install_neuronx_cc_hook()
        partition_name = (
            nc.partition_id_tensor.name if nc.partition_id_tensor else None
        )
        in_names, out_names, out_avals = [], [], []
        for alloc in nc.m.functions[0].allocations:
            if not isinstance(alloc, mybir.MemoryLocationSet):
                continue
            name = alloc.memorylocations[0].name
            if alloc.kind == "ExternalInput":
                if name != partition_name:
                    in_names.append(name)
            elif alloc.kind == "ExternalOutput":
                out_names.append(name)
                out_avals.append(
                    jax.core.ShapedArray(
                        tuple(alloc.tensor_shape), mybir.dt.np(alloc.dtype)
                    )
                )
        self.in_names, self.out_names, self.out_avals = in_names, out_names, out_avals
        n_params, n_outs = len(in_names), len(out_avals)
        all_in_names = list(in_names) + list(out_names)
        if partition_name is not None:
            all_in_names.append(partition_name)

        def _body(*args):
            operands = list(args)
            if partition_name is not None:
                operands.append(bass2jax.partition_id_tensor())
            return tuple(
                bass2jax._bass_exec_p.bind(
                    *operands,
                    out_avals=tuple(out_avals),
                    in_names=tuple(all_in_names),
                    out_names=tuple(out_names),
                    lowering_input_output_aliases=(),
                    sim_require_finite=True,
                    sim_require_nnan=True,
                    nc=nc,
                )
            )

        devices = jax.devices()[:n_cores]
        self.mesh = Mesh(np.asarray(devices), ("core",))
        self.sh = NamedSharding(self.mesh, PartitionSpec("core"))
        self.fn = jax.jit(
            shard_map(
                _body,
                mesh=self.mesh,
                in_specs=(PartitionSpec("core"),) * (n_params + n_outs),
                out_specs=(PartitionSpec("core"),) * n_outs,
                check_rep=False,
            ),
            donate_argnums=tuple(range(n_params, n_params + n_outs)),
            keep_unused=True,
        )
        self._zero_shapes = [
            ((n_cores * a.shape[0], *a.shape[1:]), a.dtype) for a in out_avals
        ]

    def put_inputs(self, in_maps):
        self.dev_in = [
            self.jax.device_put(
                np.concatenate([np.asarray(m[name]) for m in in_maps], axis=0),
                self.sh,
            )
            for name in self.in_names
        ]

    def _zeros(self):
        return [
            self.jax.device_put(np.zeros(s, d), self.sh)
            for s, d in self._zero_shapes
        ]

    def execute(self):
        outs = self.fn(*self.dev_in, *self._zeros())
        self.jax.block_until_ready(outs)
        return self.unpack(outs)

    def unpack(self, outs):
        return [
            {
                name: np.asarray(outs[i]).reshape(
                    self.n_cores, *self.out_avals[i].shape
                )[c]
                for i, name in enumerate(self.out_names)
            }
            for c in range(self.n_cores)
        ]


_RUNNERS = {}
_LAST_FALLBACKS = 0
_FORCE_FALLBACK = False  # test hook: exercise the exact host fallback path


def _get_runner(nc):
    r = _RUNNERS.get(id(nc))
    if r is None:
        r = _Runner(nc)
        _RUNNERS[id(nc)] = r
    return r


def _make_in_maps(cenT_shards, feats_p):
    ident = np.eye(128, dtype=np.float32)
    fin = np.ascontiguousarray(feats_p.reshape(RT, 128, D), dtype=np.float32)
    return [
        {
            "cenT": np.ascontiguousarray(
                cenT_shards[k].reshape(2, 128, P_LOCAL), dtype=MM_NP
            ),
            "feats": fin,
            "ident": ident,
        }
        for k in range(NCORES)
    ]


def _host_finish(results, feats_p, labels_p, cams_p, centers, tile_cams):
    # candidates come back UNNORMALIZED (raw feats dot centers); rescale by
    # 1/||f|| per row (order within a row is unaffected by the positive scale)
    invn = 1.0 / np.linalg.norm(feats_p.astype(np.float64), axis=1)
    cand = np.stack(
        [results[k]["cand"].reshape(N, CAND) for k in range(NCORES)]
    ).astype(np.float64) * invn[None, :, None]  # [8, 512, CAND]
    rows = np.arange(N)
    # srow slots: per row-tile, slot idx corresponds to tile_cams order
    slot = np.zeros(N, dtype=np.int64)
    for rt in range(RT):
        for idx, cam in enumerate(tile_cams[rt]):
            sel = slice(128 * rt, 128 * (rt + 1))
            slot[sel] = np.where(cams_p[sel] == cam, idx, slot[sel])
    rt_of = rows // 128
    p_of = rows % 128
    s_k = np.stack(
        [
            results[k]["srow"].reshape(RT, 128, C)[rt_of, p_of, slot]
            for k in range(NCORES)
        ]
    ).astype(np.float64)  # [8, 512], sum_l exp(20 * sims_intra) per core

    fe = feats_p.astype(np.float64)
    fn = fe / np.linalg.norm(fe, axis=1, keepdims=True)
    cen = centers.astype(np.float64)

    # positives: 8 same-label proxies per row (host, f64)
    gidx = labels_p[:, None] * C + np.arange(C)[None, :]        # [512, 8]
    g = cen[gidx]                                               # [512, 8, 256]
    pos = np.einsum("rcd,rd->rc", g, fn)                        # [512, 8]

    # ---- intra ----
    lse_intra = np.log(s_k.sum(axis=0))
    v = pos[np.arange(N), cams_p]
    loss_intra_i = lse_intra - INV_T * v

    # ---- inter: merge candidates, remove positive columns by value ----
    CR = cand.transpose(1, 0, 2).reshape(N, NCORES * CAND).astype(np.float64)
    owner = labels_p // L_LOCAL
    lloc = labels_p % L_LOCAL
    col0 = C * lloc                                             # local column of 1st positive
    ch0 = col0 // 1024  # 8-col positive group never straddles a 1024 boundary
    ch1 = (col0 + C - 1) // 1024
    eps = 1e-5
    for i in rows:
        base = owner[i] * CAND
        chunks = {ch0[i], ch1[i]}
        idxs = np.concatenate([np.arange(base + 8 * ch, base + 8 * ch + 8)
                               for ch in sorted(chunks)])
        vals = CR[i, idxs]
        used = np.zeros(len(idxs), bool)
        for pv in pos[i]:
            d = np.abs(vals - pv)
            d[used] = np.inf
            j = np.argmin(d)
            if d[j] < eps:
                used[j] = True
        CR[i, idxs[used]] = -np.inf

    part = np.partition(CR, NCORES * CAND - K, axis=1)[:, -K:]  # top-50 values
    t50 = part.min(axis=1)

    # certificate: every chunk's 8th-largest (pre-removal) must be <= t50
    chunk8 = cand[:, :, 7::8]                                   # [8, 512, 8]
    if _FORCE_FALLBACK:
        bad = rows
    else:
        bad = np.where(chunk8.max(axis=(0, 2)) > t50)[0]
    global _LAST_FALLBACKS
    _LAST_FALLBACKS = len(bad)
    for i in bad:
        sims_row = cen @ fn[i]                                  # [64000] exact
        sims_row[C * labels_p[i] : C * labels_p[i] + C] = -np.inf
        part[i] = np.sort(sims_row)[-K:]

    z = np.concatenate([pos, part], axis=1) * INV_T             # [512, 58]
    mz = z.max(axis=1)
    lse_inter = np.log(np.exp(z - mz[:, None]).sum(axis=1)) + mz
    loss_inter_i = lse_inter - INV_T * pos.mean(axis=1)

    # ---- per-camera means, summed ----
    cnt = np.bincount(cams_p, minlength=C).astype(np.float64)
    s_intra = np.bincount(cams_p, weights=loss_intra_i, minlength=C)
    s_inter = np.bincount(cams_p, weights=loss_inter_i, minlength=C)
    safe = np.maximum(cnt, 1.0)
    li = np.sum(np.where(cnt > 0, s_intra / safe, 0.0))
    le = LW * np.sum(np.where(cnt > 0, s_inter / safe, 0.0))
    return np.array([li, le], dtype=np.float32)


def _prepare(feats, indexes, label_table, cam_table, centers):
    feats = np.asarray(feats, dtype=np.float32)
    indexes = np.asarray(indexes)
    label_table = np.asarray(label_table)
    cam_table = np.asarray(cam_table)
    centers = np.asarray(centers, dtype=np.float32)

    labels = np.asarray(label_table[indexes], dtype=np.int64)
    cams = np.asarray(cam_table[indexes], dtype=np.int64)

    # permute rows so camera groups are contiguous, ordered big+small so most
    # 128-row tiles span only ~2 cameras (fewer intra exp instructions)
    sizes = np.bincount(cams, minlength=C)
    order = _pair_order(sizes)
    perm = np.concatenate([np.where(cams == c)[0] for c in order])
    feats_p = np.ascontiguousarray(feats[perm])
    labels_p = labels[perm]
    cams_p = cams[perm]
    tile_cams = tuple(
        tuple(dict.fromkeys(cams_p[128 * rt : 128 * (rt + 1)].tolist()))
        for rt in range(RT)
    )
    cenT_shards = [
        np.ascontiguousarray(centers[k * P_LOCAL : (k + 1) * P_LOCAL].T)
        for k in range(NCORES)
    ]
    return centers, tile_cams, feats_p, labels_p, cams_p, cenT_shards


def kernel(feats, indexes, label_table, cam_table, centers):
    centers, tile_cams, feats_p, labels_p, cams_p, cenT_shards = _prepare(
        feats, indexes, label_table, cam_table, centers
    )
    nc = _build_program(tile_cams)
    runner = _get_runner(nc)
    runner.put_inputs(_make_in_maps(cenT_shards, feats_p))
    results = runner.execute()
    return _host_finish(results, feats_p, labels_p, cams_p, centers, tile_cams)



# revision 6
# speedup vs baseline: 1.1787x; 1.1787x over previous
"""Trainium2 Bass kernel for nn_CAPMemory (camera-aware proxy memory loss).

Strategy (8 NeuronCores, SPMD, no collectives):
  - Shard the 64000x256 proxy table over P: core k owns labels
    [1000k, 1000(k+1)), all 8 cameras. Per-core column layout is CAM-MAJOR,
    padded to 4 PSUM-bank groups of 2048 columns: group g = [cam 2g (1000
    cols) | cam 2g+1 (1000 cols) | 48 zero-pad cols]. Pad sims are 0 and can
    never reach the top-50; the intra exp never reads them.
  - Matmuls run in fp8(e4m3) DoubleRow mode: operands laid out [128, 2, free]
    so one matmul contracts all K=256 at 2 MACs/cell/cycle. Centers are
    pre-scaled by 16 on the host so their entries (~N(0,1/256)) sit in e4m3's
    normal range; the 1/16 rides in the host post-scale and the exp scale.
    feats are transposed/quantized on the host (fT input); row norms arrive
    as the sc20 input. Centers SBUF is double-buffered so the fp8 DMA of the
    next iteration hides under compute.
  - Each [128, 2048] f32 PSUM group is drained by per-chunk paths chosen
    statically (host+device share the plan) to balance ACT and DVE:
      exp    : ACT exp(sc20*sims) -> bf16 image + accum_out (the intra
               denominator, needed anyway). exp is monotone, so the image's
               top-8 are the slab's candidates (exp domain).
      copy   : ACT copies the chunk (1000 or merged 2048 cols) to bf16 SBUF.
      direct : DVE InstMax top-8 straight from PSUM (exact chunk top-8).
    bf16 images are folded by DVE pairwise tensor_max (2x bf16 mode) and
    finished with InstMax. InstMax/tensor_reduce have no 2x uops, so the
    fold+InstMax split is the cheapest DVE composition; copies route part of
    the scan to ACT, 'direct' keeps the rest on DVE.
  - Candidates: top-8 per chunk, 8 value-slots per (row-tile, group) pair
    = up to 512/row global. Folded chunks can miss a top-50 element that
    shares a fold stripe with a larger one; on this data that biases the
    final scalars ~1e-4 relative (gate 2e-2). Rows whose chunk 8th-largest
    exceeds the merged t50 are recomputed exactly on the host.
  - Host merge: intra logsumexp = log(sum_k srow_k); positives in f64;
    positive candidates removed per-chunk by value-matching against an
    fp8-simulated prediction of the device value; top-50 from the merged
    candidates; per-camera means as in the reference.
"""

import os
import sys
import functools

sys.path.insert(0, "/opt/trn_rl_repo")

import numpy as np

from concourse import bacc, mybir
from concourse.tile import TileContext

F32 = mybir.dt.float32
BF16 = mybir.dt.bfloat16
FP8 = mybir.dt.float8e4
NP_FP8 = mybir.dt.np(FP8)
NP_BF16 = mybir.dt.np(BF16)

N = 512          # batch
D = 256          # feature dim
L = 8000         # labels
C = 8            # cameras
NCORES = 8
L_LOCAL = 1000   # labels per core
RT = 4           # row tiles of 128
GROUPS = 4       # PSUM groups per row tile
GW = 2048        # columns per group (2 cams * 1000 + 48 pad)
PL = GROUPS * GW # padded per-core columns (8192)
SW = 1000        # slab width (one camera's columns)
INV_T = 20.0     # 1 / temperature
K = 50           # hard negatives
LW = 0.5         # inter-cam loss weight
CEN_SCALE = 16.0 # host pre-scale on centers (keeps fp8 in normal range)
CAND_PER_S = 8
SLABS = 2 * GROUPS
CAND = SLABS * CAND_PER_S    # 64 candidate values per row per core

# experiment knobs (defaults are the shipped config)
MM = os.environ.get("V2_MM", "fp8dr")            # fp8dr|bf16
N_COPY = int(os.environ.get("V2_COPY", "19"))    # no-exp slab-equivalents via ACT copy
FOLDS_TGT = int(os.environ.get("V2_FOLDS_TGT", "256"))  # fold images down to <= this


def _pair_order(sizes):
    """Order cameras big+small so most 128-row tiles span only 2 cameras."""
    desc = np.argsort(-np.asarray(sizes), kind="stable")
    big, small = desc[: C // 2], desc[C // 2 :][::-1]
    order = []
    for b, s in zip(big, small):
        order += [int(b), int(s)]
    return order


def _plan(tile_cams):
    """Chunk plan shared by device build and host decode.

    Returns plan[rt][g] = list of (kind, h0, w, slot):
      kind in {'exp','copy','direct'}; the chunk covers group columns
      [h0*1000, h0*1000+w) and writes candidate slot `slot` (0..7 per rt).
    Groups with no exp slab are merged into one 2048-col chunk. Of the
    no-exp chunks, ~N_COPY slab-equivalents (spread evenly) go via ACT copy.
    """
    units = []   # (rt, g, h0, w, slot) no-exp chunks in emission order
    plan = [[[] for _ in range(GROUPS)] for _ in range(RT)]
    for rt in range(RT):
        for g in range(GROUPS):
            present = [h for h in range(2) if (2 * g + h) in tile_cams[rt]]
            for h in present:
                plan[rt][g].append(("exp", h, SW, 2 * g + h))
            absent = [h for h in range(2) if h not in present]
            if len(absent) == 2:
                units.append((rt, g, 0, GW, 2 * g))
            elif len(absent) == 1:
                h = absent[0]
                units.append((rt, g, h, SW, 2 * g + h))
    total = sum(u[3] for u in units) / SW
    ratio = min(1.0, N_COPY / max(total, 1e-9))
    acc = copied = 0.0
    for rt, g, h0, w, slot in units:
        acc += w / SW
        kind = "copy" if copied < ratio * acc - 1e-9 else "direct"
        if kind == "copy":
            copied += w / SW
        plan[rt][g].append((kind, h0, w, slot))
    return plan


@functools.lru_cache(maxsize=8)
def _build_program(tile_cams, repeats=1):
    nc = bacc.Bacc(None, target_bir_lowering=False, num_swdge_queues=4)

    mm_dt = FP8 if MM == "fp8dr" else BF16
    cenT = nc.dram_tensor("cenT", [128, 2, PL], mm_dt, kind="ExternalInput")
    fTd = nc.dram_tensor("fT", [RT, 128, 2, 128], mm_dt, kind="ExternalInput")
    sc20d = nc.dram_tensor("sc20", [128, RT], F32, kind="ExternalInput")
    candd = nc.dram_tensor("cand", [RT, 128, CAND], F32, kind="ExternalOutput")
    srowd = nc.dram_tensor("srow", [RT, 128, C], F32, kind="ExternalOutput")

    with TileContext(nc) as tc:
        with (
            tc.tile_pool(name="cen", bufs=2) as cenp,
            tc.tile_pool(name="ftp", bufs=2) as ftp,
            tc.tile_pool(name="m1p", bufs=3) as m1p,
            tc.tile_pool(name="smallp", bufs=2) as smallp,
            tc.tile_pool(name="outp", bufs=2) as outp,
            tc.tile_pool(name="psum", bufs=2, space="PSUM") as psump,
        ):
            for _rep in range(repeats):
                _kernel_body(nc, tc, cenp, ftp, m1p, smallp, outp, psump,
                             cenT, fTd, sc20d, candd, srowd, tile_cams)

    nc.compile()
    return nc


def _fold_and_max(nc, m1p, co, img):
    """DVE: pairwise tensor_max folds (2x bf16 mode) down to <=FTGT stripe
    maxima, then InstMax top-8."""
    cur, w = img, img.shape[1]
    while w > FOLDS_TGT:
        half = w // 2
        nxt = m1p.tile([128, half], BF16, name="fold")
        nc.vector.tensor_max(nxt[:, :], cur[:, 0:half], cur[:, half : 2 * half])
        cur, w = nxt, half
    nc.vector.max(co, cur[:, 0:w])


def _kernel_body(nc, tc, cenp, ftp, m1p, smallp, outp, psump,
                 cenT, fTd, sc20d, candd, srowd, tile_cams):
    ActF = mybir.ActivationFunctionType
    mm_dt = FP8 if MM == "fp8dr" else BF16
    plan = _plan(tile_cams)

    # small transfers first; warm the Exp LUT in ACT's idle window
    sc20_sb = smallp.tile([128, RT], F32, name="sc20", bufs=2)
    nc.sync.dma_start(out=sc20_sb[:, :], in_=sc20d[:, :])
    warm = smallp.tile([128, 1], F32, name="warm", bufs=2)
    nc.scalar.activation(warm[:, 0:1], sc20_sb[:, 0:1], ActF.Exp)

    fTs = []
    for rt in range(RT):
        fT = ftp.tile([128, 2, 128], mm_dt, name=f"fT{rt}")
        nc.scalar.dma_start(out=fT[:, :, :], in_=fTd[rt])
        fTs.append(fT)

    # centers: group-major so group-0 matmuls unblock first; spread issue
    # across engines/queues
    cen_sb = cenp.tile([128, 2, PL], mm_dt, name="cen")
    dma_engines = [nc.sync, nc.gpsimd]
    for g in range(GROUPS):
        s = slice(g * GW, (g + 1) * GW)
        for j in range(2):
            eng = dma_engines[(2 * g + j) % len(dma_engines)]
            eng.dma_start(out=cen_sb[:, j, s], in_=cenT[:, j, s])

    cand_ts = [
        outp.tile([128, CAND], F32, name=f"cand{rt}", bufs=2) for rt in range(RT)
    ]
    s_ts = [
        smallp.tile([128, C], F32, name=f"s_t{rt}", bufs=2) for rt in range(RT)
    ]

    for rt in range(RT):
        for g in range(GROUPS):
            ps = psump.tile([128, 4, 512], F32, name="ps")
            for mk in range(4):
                lo = g * GW + mk * 512
                if MM == "fp8dr":
                    nc.tensor.matmul(
                        ps[:, mk, :], fTs[rt][:, :, :],
                        cen_sb[:, :, lo : lo + 512],
                        start=True, stop=True,
                        perf_mode=mybir.MatmulPerfMode.DoubleRow,
                    )
                else:
                    nc.tensor.matmul(
                        ps[:, mk, :], fTs[rt][:, 0, :],
                        cen_sb[:, 0, lo : lo + 512],
                        start=True, stop=False,
                    )
                    nc.tensor.matmul(
                        ps[:, mk, :], fTs[rt][:, 1, :],
                        cen_sb[:, 1, lo : lo + 512],
                        start=False, stop=True,
                    )

            flat = ps.rearrange("p a b -> p (a b)")
            for kind, h0, w, slot in plan[rt][g]:
                cols = flat[:, h0 * SW : h0 * SW + w]
                co = cand_ts[rt][:, slot * CAND_PER_S : (slot + 1) * CAND_PER_S]
                if kind == "exp":
                    idx = tile_cams[rt].index(2 * g + h0)
                    scr = m1p.tile([128, w], BF16, name="scr")
                    nc.scalar.activation(
                        scr[:, :], cols, ActF.Exp,
                        scale=sc20_sb[:, rt : rt + 1],
                        accum_out=s_ts[rt][:, idx : idx + 1],
                    )
                    _fold_and_max(nc, m1p, co, scr)
                elif kind == "copy":
                    scr = m1p.tile([128, w], BF16, name="scr")
                    nc.scalar.copy(scr[:, :], cols)
                    _fold_and_max(nc, m1p, co, scr)
                else:
                    nc.vector.max(co, cols)

        nc.sync.dma_start(out=candd[rt], in_=cand_ts[rt][:, :])
        nc.sync.dma_start(out=srowd[rt], in_=s_ts[rt][:, :])


class _Runner:
    """Sharded 8-core executor for a built Bass program (axon/PJRT path)."""

    def __init__(self, nc, n_cores=NCORES):
        import jax
        from jax.sharding import Mesh, PartitionSpec, NamedSharding
        from jax.experimental.shard_map import shard_map
        from concourse import bass2jax

        self.jax = jax
        self.nc = nc
        self.n_cores = n_cores
        bass2jax.install_neuronx_cc_hook()
        partition_name = (
            nc.partition_id_tensor.name if nc.partition_id_tensor else None
        )
        in_names, out_names, out_avals = [], [], []
        for alloc in nc.m.functions[0].allocations:
            if not isinstance(alloc, mybir.MemoryLocationSet):
                continue
            name = alloc.memorylocations[0].name
            if alloc.kind == "ExternalInput":
                if name != partition_name:
                    in_names.append(name)
            elif alloc.kind == "ExternalOutput":
                out_names.append(name)
                out_avals.append(
                    jax.core.ShapedArray(
                        tuple(alloc.tensor_shape), mybir.dt.np(alloc.dtype)
                    )
                )
        self.in_names, self.out_names, self.out_avals = in_names, out_names, out_avals
        n_params, n_outs = len(in_names), len(out_avals)
        all_in_names = list(in_names) + list(out_names)
        if partition_name is not None:
            all_in_names.append(partition_name)

        def _body(*args):
            operands = list(args)
            if partition_name is not None:
                operands.append(bass2jax.partition_id_tensor())
            return tuple(
                bass2jax._bass_exec_p.bind(
                    *operands,
                    out_avals=tuple(out_avals),
                    in_names=tuple(all_in_names),
                    out_names=tuple(out_names),
                    lowering_input_output_aliases=(),
                    sim_require_finite=True,
                    sim_require_nnan=True,
                    nc=nc,
                )
            )

        devices = jax.devices()[:n_cores]
        self.mesh = Mesh(np.asarray(devices), ("core",))
        self.sh = NamedSharding(self.mesh, PartitionSpec("core"))
        self.fn = jax.jit(
            shard_map(
                _body,
                mesh=self.mesh,
                in_specs=(PartitionSpec("core"),) * (n_params + n_outs),
                out_specs=(PartitionSpec("core"),) * n_outs,
                check_rep=False,
            ),
            donate_argnums=tuple(range(n_params, n_params + n_outs)),
            keep_unused=True,
        )
        self._zero_shapes = [
            ((n_cores * a.shape[0], *a.shape[1:]), a.dtype) for a in out_avals
        ]

    def put_inputs(self, in_maps):
        self.dev_in = [
            self.jax.device_put(
                np.concatenate([np.asarray(m[name]) for m in in_maps], axis=0),
                self.sh,
            )
            for name in self.in_names
        ]

    def _zeros(self):
        return [
            self.jax.device_put(np.zeros(s, d), self.sh)
            for s, d in self._zero_shapes
        ]

    def execute(self):
        outs = self.fn(*self.dev_in, *self._zeros())
        self.jax.block_until_ready(outs)
        return self.unpack(outs)

    def unpack(self, outs):
        return [
            {
                name: np.asarray(outs[i]).reshape(
                    self.n_cores, *self.out_avals[i].shape
                )[c]
                for i, name in enumerate(self.out_names)
            }
            for c in range(self.n_cores)
        ]


_RUNNERS = {}
_LAST_FALLBACKS = 0
_FORCE_FALLBACK = False  # test hook: exercise the exact host fallback path


def _get_runner(nc):
    r = _RUNNERS.get(id(nc))
    if r is None:
        r = _Runner(nc)
        _RUNNERS[id(nc)] = r
    return r


def _make_in_maps(cenT_shards, feats_p):
    np_mm = NP_FP8 if MM == "fp8dr" else NP_BF16
    inv = 1.0 / np.linalg.norm(feats_p.astype(np.float64), axis=1)
    sc20 = np.ascontiguousarray(
        (INV_T / CEN_SCALE) * inv.reshape(RT, 128).T, dtype=np.float32
    )  # [128, RT]
    # fT[rt, p, j, m] = feats_p[rt*128 + m, 128*j + p]
    fT = np.ascontiguousarray(
        feats_p.reshape(RT, 128, 2, 128).transpose(0, 3, 2, 1), dtype=np_mm
    )
    return [
        {"cenT": cenT_shards[k], "fT": fT, "sc20": sc20}
        for k in range(NCORES)
    ]


def _host_finish(results, feats_p, labels_p, cams_p, centers, tile_cams):
    rows = np.arange(N)
    invn = 1.0 / np.linalg.norm(feats_p.astype(np.float64), axis=1)
    plan = _plan(tile_cams)
    # chunk tables: slab (camera) -> covering chunk slot + kind, per rt
    slab_slot = np.full((RT, SLABS), -1, dtype=np.int64)
    slab_kind = [[None] * SLABS for _ in range(RT)]
    active = np.zeros((RT, SLABS), dtype=bool)     # slots that carry values
    exp_slot = np.zeros((RT, SLABS), dtype=bool)   # slot domain is exp
    for rt in range(RT):
        for g in range(GROUPS):
            for kind, h0, w, slot in plan[rt][g]:
                active[rt, slot] = True
                exp_slot[rt, slot] = kind == "exp"
                for h in (range(2) if w == GW else [h0]):
                    slab_slot[rt, 2 * g + h] = slot
                    slab_kind[rt][2 * g + h] = kind

    cand_raw = np.stack(
        [results[k]["cand"].reshape(N, SLABS, CAND_PER_S) for k in range(NCORES)]
    ).astype(np.float64)  # [8, 512, 8slots, 8]
    cscale = invn / CEN_SCALE
    rt_of = rows // 128
    is_exp = exp_slot[rt_of]                       # [512, 8slots]
    act = active[rt_of]                            # [512, 8slots]
    cand = np.where(
        is_exp[None, :, :, None],
        np.log(np.maximum(cand_raw, 1e-30)) / INV_T,
        cand_raw * cscale[None, :, None, None],
    )
    cand = np.where(act[None, :, :, None], cand, -np.inf)

    # srow slots: per row-tile, slot idx corresponds to tile_cams order
    slot = np.zeros(N, dtype=np.int64)
    for rt in range(RT):
        for idx, cam in enumerate(tile_cams[rt]):
            sel = slice(128 * rt, 128 * (rt + 1))
            slot[sel] = np.where(cams_p[sel] == cam, idx, slot[sel])
    p_of = rows % 128
    s_k = np.stack(
        [
            results[k]["srow"].reshape(RT, 128, C)[rt_of, p_of, slot]
            for k in range(NCORES)
        ]
    ).astype(np.float64)  # [8, 512]

    fe = feats_p.astype(np.float64)
    fn = fe / np.linalg.norm(fe, axis=1, keepdims=True)
    cen = centers.astype(np.float64)

    # positives: 8 same-label proxies per row (host, f64)
    gidx = labels_p[:, None] * C + np.arange(C)[None, :]        # [512, 8]
    pos = np.einsum("rcd,rd->rc", cen[gidx], fn)                # [512, 8]

    # ---- intra ----
    lse_intra = np.log(s_k.sum(axis=0))
    v = pos[np.arange(N), cams_p]
    loss_intra_i = lse_intra - INV_T * v

    # ---- inter: remove positives from candidates by value, then top-50 ----
    np_mm = NP_FP8 if MM == "fp8dr" else NP_BF16
    f_q = feats_p.astype(np_mm).astype(np.float64)
    g_q = (CEN_SCALE * centers[gidx]).astype(np_mm).astype(np.float64)
    pos_dev = np.einsum("rcd,rd->rc", g_q, f_q).astype(np.float32)  # raw dot
    sc20r = (INV_T / CEN_SCALE) * invn
    pred_exp = (
        np.log(
            np.exp(sc20r[:, None] * pos_dev.astype(np.float64))
            .astype(NP_BF16).astype(np.float64)
        ) / INV_T
    )
    pred_raw_b = pos_dev.astype(NP_BF16).astype(np.float64) * cscale[:, None]
    pred_raw_x = pos_dev.astype(np.float64) * cscale[:, None]

    CRS = cand.transpose(1, 0, 2, 3)                       # [512, 8cores, 8, 8]
    owner = labels_p // L_LOCAL
    for i in rows:
        rt = i // 128
        for c in range(C):
            kind = slab_kind[rt][c]
            sl = slab_slot[rt, c]
            if kind == "exp":
                pv = pred_exp[i, c]
            elif kind == "copy":
                pv = pred_raw_b[i, c]
            else:
                pv = pred_raw_x[i, c]
            vals = CRS[i, owner[i], sl]
            d = np.abs(vals - pv)
            j = np.argmin(d)
            if d[j] < 2.5e-4 + 5e-3 * abs(pv):
                CRS[i, owner[i], sl, j] = -np.inf

    CR = CRS.reshape(N, NCORES * CAND)
    part = np.partition(CR, NCORES * CAND - K, axis=1)[:, -K:]  # top-50 values
    t50 = part.min(axis=1)

    # at-risk check: each chunk's 8th-largest candidate should be <= t50
    # (sound certificate for 'direct' chunks, heuristic for folded chunks)
    slab8 = np.where(act[None], cand[:, :, :, CAND_PER_S - 1], -np.inf)
    if _FORCE_FALLBACK:
        bad = rows
    else:
        bad = np.where(slab8.max(axis=(0, 2)) > t50)[0]
    global _LAST_FALLBACKS
    _LAST_FALLBACKS = len(bad)
    for i in bad:
        sims_row = cen @ fn[i]                                  # [64000] exact
        sims_row[C * labels_p[i] : C * labels_p[i] + C] = -np.inf
        part[i] = np.sort(sims_row)[-K:]

    z = np.concatenate([pos, part], axis=1) * INV_T             # [512, 58]
    mz = z.max(axis=1)
    lse_inter = np.log(np.exp(z - mz[:, None]).sum(axis=1)) + mz
    loss_inter_i = lse_inter - INV_T * pos.mean(axis=1)

    # ---- per-camera means, summed ----
    cnt = np.bincount(cams_p, minlength=C).astype(np.float64)
    s_intra = np.bincount(cams_p, weights=loss_intra_i, minlength=C)
    s_inter = np.bincount(cams_p, weights=loss_inter_i, minlength=C)
    safe = np.maximum(cnt, 1.0)
    li = np.sum(np.where(cnt > 0, s_intra / safe, 0.0))
    le = LW * np.sum(np.where(cnt > 0, s_inter / safe, 0.0))
    return np.array([li, le], dtype=np.float32)


def _prepare(feats, indexes, label_table, cam_table, centers):
    feats = np.asarray(feats, dtype=np.float32)
    indexes = np.asarray(indexes)
    label_table = np.asarray(label_table)
    cam_table = np.asarray(cam_table)
    centers = np.asarray(centers, dtype=np.float32)

    labels = np.asarray(label_table[indexes], dtype=np.int64)
    cams = np.asarray(cam_table[indexes], dtype=np.int64)

    # permute rows so camera groups are contiguous, ordered big+small so most
    # 128-row tiles span only ~2 cameras (fewer intra exp instructions)
    sizes = np.bincount(cams, minlength=C)
    order = _pair_order(sizes)
    perm = np.concatenate([np.where(cams == c)[0] for c in order])
    feats_p = np.ascontiguousarray(feats[perm])
    labels_p = labels[perm]
    cams_p = cams[perm]
    tile_cams = tuple(
        tuple(dict.fromkeys(cams_p[128 * rt : 128 * (rt + 1)].tolist()))
        for rt in range(RT)
    )

    # per-core centers, cam-major with 48-col pad per group, pre-scaled,
    # transposed to [128, 2, PL] (partition=feature_lo, j=feature_hi)
    np_mm = NP_FP8 if MM == "fp8dr" else NP_BF16
    by_cam = centers.reshape(L, C, D)
    cenT_shards = []
    for k in range(NCORES):
        X = by_cam[k * L_LOCAL : (k + 1) * L_LOCAL]             # [1000, 8, 256]
        CP = np.zeros((GROUPS, GW, D), dtype=np.float32)
        for g in range(GROUPS):
            CP[g, 0:1000] = X[:, 2 * g, :]
            CP[g, 1000:2000] = X[:, 2 * g + 1, :]
        CP = (CEN_SCALE * CP).reshape(PL, 2, 128)
        cenT_shards.append(
            np.ascontiguousarray(CP.transpose(2, 1, 0), dtype=np_mm)
        )
    return centers, tile_cams, feats_p, labels_p, cams_p, cenT_shards


def kernel(feats, indexes, label_table, cam_table, centers):
    centers, tile_cams, feats_p, labels_p, cams_p, cenT_shards = _prepare(
        feats, indexes, label_table, cam_table, centers
    )
    nc = _build_program(tile_cams)
    runner = _get_runner(nc)
    runner.put_inputs(_make_in_maps(cenT_shards, feats_p))
    results = runner.execute()
    return _host_finish(results, feats_p, labels_p, cams_p, centers, tile_cams)


# revision 13
# speedup vs baseline: 1.3989x; 1.1868x over previous
"""Trainium2 Bass kernel for nn_CAPMemory (camera-aware proxy memory loss).

Strategy (8 NeuronCores, SPMD, no collectives):
  - Shard the 64000x256 proxy table over P: core k owns labels
    [1000k, 1000(k+1)), all 8 cameras. Per-core column layout is CAM-MAJOR,
    padded to 4 PSUM-bank groups of 2048 columns: group g = [cam 2g (1000
    cols) | cam 2g+1 (1000 cols) | 48 zero-pad cols]. Pad sims are 0 and can
    never reach the top-50; the intra exp never reads them.
  - Matmuls run in fp8(e4m3) DoubleRow mode: operands laid out [128, 2, free]
    so one matmul contracts all K=256 at 2 MACs/cell/cycle. Centers are
    pre-scaled by 16 on the host so their entries (~N(0,1/256)) sit in e4m3's
    normal range; the 1/16 rides in the host post-scale and the exp scale.
    feats are transposed/quantized on the host (fT input); row norms arrive
    as the sc20 input. Centers SBUF is double-buffered so the fp8 DMA of the
    next iteration hides under compute.
  - Each [128, 2048] f32 PSUM group is drained by per-chunk paths chosen
    statically (host+device share the plan) to balance ACT and DVE:
      exp    : ACT exp(sc20*sims) -> bf16 image + accum_out (the intra
               denominator, needed anyway). exp is monotone, so the image's
               top-8 are the slab's candidates (exp domain).
      copy   : ACT copies the chunk (1000 or merged 2048 cols) to bf16 SBUF.
      direct : DVE InstMax top-8 straight from PSUM (exact chunk top-8).
    bf16 images are folded by DVE pairwise tensor_max (2x bf16 mode) and
    finished with InstMax. InstMax/tensor_reduce have no 2x uops, so the
    fold+InstMax split is the cheapest DVE composition; copies route part of
    the scan to ACT, 'direct' keeps the rest on DVE.
  - Candidates: top-8 per chunk, 8 value-slots per (row-tile, group) pair
    = up to 512/row global. Folded chunks can miss a top-50 element that
    shares a fold stripe with a larger one; on this data that biases the
    final scalars ~1e-4 relative (gate 2e-2). Rows whose chunk 8th-largest
    exceeds the merged t50 are recomputed exactly on the host.
  - Host merge: intra logsumexp = log(sum_k srow_k); positives in f64;
    positive candidates removed per-chunk by value-matching against an
    fp8-simulated prediction of the device value; top-50 from the merged
    candidates; per-camera means as in the reference.
"""

import os
import sys
import functools

sys.path.insert(0, "/opt/trn_rl_repo")

import numpy as np

from concourse import bacc, mybir
from concourse.tile import TileContext

F32 = mybir.dt.float32
BF16 = mybir.dt.bfloat16
FP8 = mybir.dt.float8e4
NP_FP8 = mybir.dt.np(FP8)
NP_BF16 = mybir.dt.np(BF16)

N = 512          # batch
D = 256          # feature dim
L = 8000         # labels
C = 8            # cameras
NCORES = 8
L_LOCAL = 1000   # labels per core
RT = 4           # row tiles of 128
GROUPS = 4       # PSUM groups per row tile
GW = 2048        # columns per group (2 cams * 1000 + 48 pad)
PL = GROUPS * GW # padded per-core columns (8192)
SW = 1000        # slab width (one camera's columns)
INV_T = 20.0     # 1 / temperature
K = 50           # hard negatives
LW = 0.5         # inter-cam loss weight
CEN_SCALE = 16.0 # host pre-scale on centers (keeps fp8 in normal range)
CAND_PER_S = 8
SLABS = 2 * GROUPS
CAND = SLABS * CAND_PER_S    # 64 candidate values per row per core

# experiment knobs (defaults are the shipped config)
MM = os.environ.get("V2_MM", "fp8dr")            # fp8dr|bf16
N_COPY = int(os.environ.get("V2_COPY", "19"))    # no-exp slab-equivalents via ACT copy
FOLDS_TGT = int(os.environ.get("V2_FOLDS_TGT", "256"))  # fold images down to <= this
M1BUFS = int(os.environ.get("V2_M1BUFS", "3"))   # scr/fold tile ring depth


def _pair_order(sizes):
    """Order cameras big+small so most 128-row tiles span only 2 cameras."""
    desc = np.argsort(-np.asarray(sizes), kind="stable")
    big, small = desc[: C // 2], desc[C // 2 :][::-1]
    order = []
    for b, s in zip(big, small):
        order += [int(b), int(s)]
    return order


def _plan(tile_cams):
    """Chunk plan shared by device build and host decode.

    Returns plan[rt] = list of chunk descriptors:
      ('direct', [(g, h0, w)], slot)            DVE InstMax straight from PSUM
      ('img', domain, [(g, h0, w, kind)...], slot)
          1-2 writers ('exp' or 'copy') fill one bf16 image, which is folded
          on DVE and finished with one InstMax into candidate slot `slot`.
    domain is 'exp' or 'raw'; a slot covers all its writers' slabs. Groups
    with two no-exp slabs form one 2048-col unit. Of the no-exp units,
    ~N_COPY slab-equivalents (spread evenly) go via the ACT-copy image path;
    same-domain images within a row tile are paired to share fold chains.
    """
    plan = []
    for rt in range(RT):
        chunks = []
        exp_writers = []
        units = []
        for g in range(GROUPS):
            present = [h for h in range(2) if (2 * g + h) in tile_cams[rt]]
            for h in present:
                exp_writers.append((g, h, SW, "exp"))
            absent = [h for h in range(2) if h not in present]
            if len(absent) == 2:
                units.append([(g, 0, GW)])
            elif len(absent) == 1:
                units.append([(g, absent[0], SW)])
        plan.append((chunks, exp_writers, units))

    # assign copy/direct over all no-exp units (N_COPY slab-equivalents)
    all_units = [(rt, u) for rt in range(RT) for u in plan[rt][2]]
    total = sum(u[0][2] for _, u in all_units) / SW
    ratio = min(1.0, N_COPY / max(total, 1e-9))
    acc = copied = 0.0
    unit_kind = []
    for _, u in all_units:
        acc += u[0][2] / SW
        k = "copy" if copied < ratio * acc - 1e-9 else "direct"
        if k == "copy":
            copied += u[0][2] / SW
        unit_kind.append(k)

    out = []
    ui = 0
    for rt in range(RT):
        chunks, exp_writers, units = plan[rt]
        copy_writers = []
        for u in units:
            k = unit_kind[ui]
            ui += 1
            if k == "direct":
                g, h0, w = u[0]
                chunks.append(("direct", g, h0, w, 2 * g + h0))
            else:
                for (g, h0, w) in u:
                    copy_writers.append((g, h0, w, "copy"))
        # pair same-domain writers (consecutive) to share fold chains
        for writers, dom in ((exp_writers, "exp"), (copy_writers, "raw")):
            for i in range(0, len(writers), 2):
                grp = writers[i : i + 2]
                slot = 2 * grp[0][0] + grp[0][1]
                chunks.append(("img", dom, grp, slot))
        out.append(chunks)
    return out


@functools.lru_cache(maxsize=8)
def _build_program(tile_cams, repeats=1):
    nc = bacc.Bacc(None, target_bir_lowering=False, num_swdge_queues=4)

    mm_dt = FP8 if MM == "fp8dr" else BF16
    cenT = nc.dram_tensor("cenT", [128, 2, PL], mm_dt, kind="ExternalInput")
    fTd = nc.dram_tensor("fT", [RT, 128, 2, 128], mm_dt, kind="ExternalInput")
    sc20d = nc.dram_tensor("sc20", [128, RT], F32, kind="ExternalInput")
    candd = nc.dram_tensor("cand", [RT, 128, CAND], F32, kind="ExternalOutput")
    srowd = nc.dram_tensor("srow", [RT, 128, C], F32, kind="ExternalOutput")

    with TileContext(nc) as tc:
        with (
            tc.tile_pool(name="cen", bufs=2) as cenp,
            tc.tile_pool(name="ftp", bufs=2) as ftp,
            tc.tile_pool(name="m1p", bufs=M1BUFS) as m1p,
            tc.tile_pool(name="smallp", bufs=2) as smallp,
            tc.tile_pool(name="outp", bufs=2) as outp,
            tc.tile_pool(name="psum", bufs=2, space="PSUM") as psump,
        ):
            for _rep in range(repeats):
                _kernel_body(nc, tc, cenp, ftp, m1p, smallp, outp, psump,
                             cenT, fTd, sc20d, candd, srowd, tile_cams)

    nc.compile()
    return nc


def _fold_and_max(nc, m1p, co, img):
    """DVE: pairwise tensor_max folds (2x bf16 mode) down to <=FTGT stripe
    maxima, then InstMax top-8."""
    cur, w = img, img.shape[1]
    while w > FOLDS_TGT and w % 2 == 0:
        half = w // 2
        nxt = m1p.tile([128, half], BF16, name="fold")
        nc.vector.tensor_max(nxt[:, :], cur[:, 0:half], cur[:, half : 2 * half])
        cur, w = nxt, half
    nc.vector.max(co, cur[:, 0:w])


def _kernel_body(nc, tc, cenp, ftp, m1p, smallp, outp, psump,
                 cenT, fTd, sc20d, candd, srowd, tile_cams):
    ActF = mybir.ActivationFunctionType
    mm_dt = FP8 if MM == "fp8dr" else BF16
    plan = _plan(tile_cams)

    # small transfers first; warm the Exp LUT in ACT's idle window
    sc20_sb = smallp.tile([128, RT], F32, name="sc20", bufs=2)
    nc.sync.dma_start(out=sc20_sb[:, :], in_=sc20d[:, :])
    warm = smallp.tile([128, 1], F32, name="warm", bufs=2)
    nc.scalar.activation(warm[:, 0:1], sc20_sb[:, 0:1], ActF.Exp)

    fTs = []
    for rt in range(RT):
        fT = ftp.tile([128, 2, 128], mm_dt, name=f"fT{rt}")
        nc.scalar.dma_start(out=fT[:, :, :], in_=fTd[rt])
        fTs.append(fT)

    # centers: group-major so group-0 matmuls unblock first; spread issue
    # across engines/queues
    cen_sb = cenp.tile([128, 2, PL], mm_dt, name="cen")
    dma_engines = [nc.sync, nc.gpsimd]
    for g in range(GROUPS):
        s = slice(g * GW, (g + 1) * GW)
        for j in range(2):
            eng = dma_engines[(2 * g + j) % len(dma_engines)]
            eng.dma_start(out=cen_sb[:, j, s], in_=cenT[:, j, s])

    cand_ts = [
        outp.tile([128, CAND], F32, name=f"cand{rt}", bufs=2) for rt in range(RT)
    ]
    s_ts = [
        smallp.tile([128, C], F32, name=f"s_t{rt}", bufs=2) for rt in range(RT)
    ]

    for rt in range(RT):
        chunks = plan[rt]
        # images: chunk -> (tile, [(writer offset, writer)...]) with lazy alloc
        img_state = {}
        for ci, ch in enumerate(chunks):
            if ch[0] == "img":
                img_state[ci] = {"tile": None, "done": 0,
                                 "w": sum(wr[2] for wr in ch[2])}

        def _cand_slot(slot):
            return cand_ts[rt][:, slot * CAND_PER_S : (slot + 1) * CAND_PER_S]

        for g in range(GROUPS):
            ps = psump.tile([128, 4, 512], F32, name="ps")
            for mk in range(4):
                lo = g * GW + mk * 512
                if MM == "fp8dr":
                    nc.tensor.matmul(
                        ps[:, mk, :], fTs[rt][:, :, :],
                        cen_sb[:, :, lo : lo + 512],
                        start=True, stop=True,
                        perf_mode=mybir.MatmulPerfMode.DoubleRow,
                    )
                else:
                    nc.tensor.matmul(
                        ps[:, mk, :], fTs[rt][:, 0, :],
                        cen_sb[:, 0, lo : lo + 512],
                        start=True, stop=False,
                    )
                    nc.tensor.matmul(
                        ps[:, mk, :], fTs[rt][:, 1, :],
                        cen_sb[:, 1, lo : lo + 512],
                        start=False, stop=True,
                    )

            flat = ps.rearrange("p a b -> p (a b)")
            for ci, ch in enumerate(chunks):
                if ch[0] == "direct":
                    _, cg, h0, w, slot = ch
                    if cg == g:
                        nc.vector.max(_cand_slot(slot), flat[:, h0 * SW : h0 * SW + w])
                    continue
                _, dom, writers, slot = ch
                st = img_state[ci]
                off = 0
                for wg, h0, w, kind in writers:
                    if wg == g:
                        if st["tile"] is None:
                            st["tile"] = m1p.tile([128, st["w"]], BF16, name="img")
                        dst = st["tile"][:, off : off + w]
                        cols = flat[:, h0 * SW : h0 * SW + w]
                        if kind == "exp":
                            idx = tile_cams[rt].index(2 * wg + h0)
                            nc.scalar.activation(
                                dst, cols, ActF.Exp,
                                scale=sc20_sb[:, rt : rt + 1],
                                accum_out=s_ts[rt][:, idx : idx + 1],
                            )
                        else:
                            nc.scalar.copy(dst, cols)
                        st["done"] += 1
                        if st["done"] == len(writers):
                            _fold_and_max(nc, m1p, _cand_slot(slot), st["tile"])
                    off += w

        nc.sync.dma_start(out=candd[rt], in_=cand_ts[rt][:, :])
        nc.sync.dma_start(out=srowd[rt], in_=s_ts[rt][:, :])


class _Runner:
    """Sharded 8-core executor for a built Bass program (axon/PJRT path)."""

    def __init__(self, nc, n_cores=NCORES):
        import jax
        from jax.sharding import Mesh, PartitionSpec, NamedSharding
        from jax.experimental.shard_map import shard_map
        from concourse import bass2jax

        self.jax = jax
        self.nc = nc
        self.n_cores = n_cores
        bass2jax.install_neuronx_cc_hook()
        partition_name = (
            nc.partition_id_tensor.name if nc.partition_id_tensor else None
        )
        in_names, out_names, out_avals = [], [], []
        for alloc in nc.m.functions[0].allocations:
            if not isinstance(alloc, mybir.MemoryLocationSet):
                continue
            name = alloc.memorylocations[0].name
            if alloc.kind == "ExternalInput":
                if name != partition_name:
                    in_names.append(name)
            elif alloc.kind == "ExternalOutput":
                out_names.append(name)
                out_avals.append(
                    jax.core.ShapedArray(
                        tuple(alloc.tensor_shape), mybir.dt.np(alloc.dtype)
                    )
                )
        self.in_names, self.out_names, self.out_avals = in_names, out_names, out_avals
        n_params, n_outs = len(in_names), len(out_avals)
        all_in_names = list(in_names) + list(out_names)
        if partition_name is not None:
            all_in_names.append(partition_name)

        def _body(*args):
            operands = list(args)
            if partition_name is not None:
                operands.append(bass2jax.partition_id_tensor())
            return tuple(
                bass2jax._bass_exec_p.bind(
                    *operands,
                    out_avals=tuple(out_avals),
                    in_names=tuple(all_in_names),
                    out_names=tuple(out_names),
                    lowering_input_output_aliases=(),
                    sim_require_finite=True,
                    sim_require_nnan=True,
                    nc=nc,
                )
            )

        devices = jax.devices()[:n_cores]
        self.mesh = Mesh(np.asarray(devices), ("core",))
        self.sh = NamedSharding(self.mesh, PartitionSpec("core"))
        self.fn = jax.jit(
            shard_map(
                _body,
                mesh=self.mesh,
                in_specs=(PartitionSpec("core"),) * (n_params + n_outs),
                out_specs=(PartitionSpec("core"),) * n_outs,
                check_rep=False,
            ),
            donate_argnums=tuple(range(n_params, n_params + n_outs)),
            keep_unused=True,
        )
        self._zero_shapes = [
            ((n_cores * a.shape[0], *a.shape[1:]), a.dtype) for a in out_avals
        ]

    def put_inputs(self, in_maps):
        self.dev_in = [
            self.jax.device_put(
                np.concatenate([np.asarray(m[name]) for m in in_maps], axis=0),
                self.sh,
            )
            for name in self.in_names
        ]

    def _zeros(self):
        return [
            self.jax.device_put(np.zeros(s, d), self.sh)
            for s, d in self._zero_shapes
        ]

    def execute(self):
        outs = self.fn(*self.dev_in, *self._zeros())
        self.jax.block_until_ready(outs)
        return self.unpack(outs)

    def unpack(self, outs):
        return [
            {
                name: np.asarray(outs[i]).reshape(
                    self.n_cores, *self.out_avals[i].shape
                )[c]
                for i, name in enumerate(self.out_names)
            }
            for c in range(self.n_cores)
        ]


_RUNNERS = {}
_LAST_FALLBACKS = 0
_FORCE_FALLBACK = False  # test hook: exercise the exact host fallback path


def _get_runner(nc):
    r = _RUNNERS.get(id(nc))
    if r is None:
        r = _Runner(nc)
        _RUNNERS[id(nc)] = r
    return r


def _make_in_maps(cenT_shards, feats_p):
    np_mm = NP_FP8 if MM == "fp8dr" else NP_BF16
    inv = 1.0 / np.linalg.norm(feats_p.astype(np.float64), axis=1)
    sc20 = np.ascontiguousarray(
        (INV_T / CEN_SCALE) * inv.reshape(RT, 128).T, dtype=np.float32
    )  # [128, RT]
    # fT[rt, p, j, m] = feats_p[rt*128 + m, 128*j + p]
    fT = np.ascontiguousarray(
        feats_p.reshape(RT, 128, 2, 128).transpose(0, 3, 2, 1), dtype=np_mm
    )
    return [
        {"cenT": cenT_shards[k], "fT": fT, "sc20": sc20}
        for k in range(NCORES)
    ]


def _host_finish(results, feats_p, labels_p, cams_p, centers, tile_cams):
    rows = np.arange(N)
    invn = 1.0 / np.linalg.norm(feats_p.astype(np.float64), axis=1)
    plan = _plan(tile_cams)
    # chunk tables: slab (camera) -> covering chunk slot + kind, per rt
    slab_slot = np.full((RT, SLABS), -1, dtype=np.int64)
    slab_kind = [[None] * SLABS for _ in range(RT)]
    active = np.zeros((RT, SLABS), dtype=bool)     # slots that carry values
    exp_slot = np.zeros((RT, SLABS), dtype=bool)   # slot domain is exp
    for rt in range(RT):
        for ch in plan[rt]:
            if ch[0] == "direct":
                _, g, h0, w, slot = ch
                writers = [(g, h0, w, "direct")]
                dom = "raw"
            else:
                _, dom, writers, slot = ch
            active[rt, slot] = True
            exp_slot[rt, slot] = dom == "exp"
            for g, h0, w, kind in writers:
                for h in (range(2) if w == GW else [h0]):
                    slab_slot[rt, 2 * g + h] = slot
                    slab_kind[rt][2 * g + h] = kind

    cand_raw = np.stack(
        [results[k]["cand"].reshape(N, SLABS, CAND_PER_S) for k in range(NCORES)]
    ).astype(np.float64)  # [8, 512, 8slots, 8]
    cscale = invn / CEN_SCALE
    rt_of = rows // 128
    is_exp = exp_slot[rt_of]                       # [512, 8slots]
    act = active[rt_of]                            # [512, 8slots]
    cand = np.where(
        is_exp[None, :, :, None],
        np.log(np.maximum(cand_raw, 1e-30)) / INV_T,
        cand_raw * cscale[None, :, None, None],
    )
    cand = np.where(act[None, :, :, None], cand, -np.inf)

    # srow slots: per row-tile, slot idx corresponds to tile_cams order
    slot = np.zeros(N, dtype=np.int64)
    for rt in range(RT):
        for idx, cam in enumerate(tile_cams[rt]):
            sel = slice(128 * rt, 128 * (rt + 1))
            slot[sel] = np.where(cams_p[sel] == cam, idx, slot[sel])
    p_of = rows % 128
    s_k = np.stack(
        [
            results[k]["srow"].reshape(RT, 128, C)[rt_of, p_of, slot]
            for k in range(NCORES)
        ]
    ).astype(np.float64)  # [8, 512]

    fe = feats_p.astype(np.float64)
    fn = fe / np.linalg.norm(fe, axis=1, keepdims=True)
    cen = centers.astype(np.float64)

    # positives: 8 same-label proxies per row (host, f64)
    gidx = labels_p[:, None] * C + np.arange(C)[None, :]        # [512, 8]
    pos = np.einsum("rcd,rd->rc", cen[gidx], fn)                # [512, 8]

    # ---- intra ----
    lse_intra = np.log(s_k.sum(axis=0))
    v = pos[np.arange(N), cams_p]
    loss_intra_i = lse_intra - INV_T * v

    # ---- inter: remove positives from candidates by value, then top-50 ----
    np_mm = NP_FP8 if MM == "fp8dr" else NP_BF16
    f_q = feats_p.astype(np_mm).astype(np.float64)
    g_q = (CEN_SCALE * centers[gidx]).astype(np_mm).astype(np.float64)
    pos_dev = np.einsum("rcd,rd->rc", g_q, f_q).astype(np.float32)  # raw dot
    sc20r = (INV_T / CEN_SCALE) * invn
    pred_exp = (
        np.log(
            np.exp(sc20r[:, None] * pos_dev.astype(np.float64))
            .astype(NP_BF16).astype(np.float64)
        ) / INV_T
    )
    pred_raw_b = pos_dev.astype(NP_BF16).astype(np.float64) * cscale[:, None]
    pred_raw_x = pos_dev.astype(np.float64) * cscale[:, None]

    CRS = cand.transpose(1, 0, 2, 3)                       # [512, 8cores, 8, 8]
    owner = labels_p // L_LOCAL
    for i in rows:
        rt = i // 128
        for c in range(C):
            kind = slab_kind[rt][c]
            sl = slab_slot[rt, c]
            if kind == "exp":
                pv = pred_exp[i, c]
            elif kind == "copy":
                pv = pred_raw_b[i, c]
            else:
                pv = pred_raw_x[i, c]
            vals = CRS[i, owner[i], sl]
            d = np.abs(vals - pv)
            j = np.argmin(d)
            if d[j] < 2.5e-4 + 5e-3 * abs(pv):
                CRS[i, owner[i], sl, j] = -np.inf

    CR = CRS.reshape(N, NCORES * CAND)
    part = np.partition(CR, NCORES * CAND - K, axis=1)[:, -K:]  # top-50 values
    t50 = part.min(axis=1)

    # at-risk check: each chunk's 8th-largest candidate should be <= t50
    # (sound certificate for 'direct' chunks, heuristic for folded chunks)
    slab8 = np.where(act[None], cand[:, :, :, CAND_PER_S - 1], -np.inf)
    if _FORCE_FALLBACK:
        bad = rows
    else:
        bad = np.where(slab8.max(axis=(0, 2)) > t50)[0]
    global _LAST_FALLBACKS
    _LAST_FALLBACKS = len(bad)
    for i in bad:
        sims_row = cen @ fn[i]                                  # [64000] exact
        sims_row[C * labels_p[i] : C * labels_p[i] + C] = -np.inf
        part[i] = np.sort(sims_row)[-K:]

    z = np.concatenate([pos, part], axis=1) * INV_T             # [512, 58]
    mz = z.max(axis=1)
    lse_inter = np.log(np.exp(z - mz[:, None]).sum(axis=1)) + mz
    loss_inter_i = lse_inter - INV_T * pos.mean(axis=1)

    # ---- per-camera means, summed ----
    cnt = np.bincount(cams_p, minlength=C).astype(np.float64)
    s_intra = np.bincount(cams_p, weights=loss_intra_i, minlength=C)
    s_inter = np.bincount(cams_p, weights=loss_inter_i, minlength=C)
    safe = np.maximum(cnt, 1.0)
    li = np.sum(np.where(cnt > 0, s_intra / safe, 0.0))
    le = LW * np.sum(np.where(cnt > 0, s_inter / safe, 0.0))
    return np.array([li, le], dtype=np.float32)


def _prepare(feats, indexes, label_table, cam_table, centers):
    feats = np.asarray(feats, dtype=np.float32)
    indexes = np.asarray(indexes)
    label_table = np.asarray(label_table)
    cam_table = np.asarray(cam_table)
    centers = np.asarray(centers, dtype=np.float32)

    labels = np.asarray(label_table[indexes], dtype=np.int64)
    cams = np.asarray(cam_table[indexes], dtype=np.int64)

    # permute rows so camera groups are contiguous, ordered big+small so most
    # 128-row tiles span only ~2 cameras (fewer intra exp instructions)
    sizes = np.bincount(cams, minlength=C)
    order = _pair_order(sizes)
    perm = np.concatenate([np.where(cams == c)[0] for c in order])
    feats_p = np.ascontiguousarray(feats[perm])
    labels_p = labels[perm]
    cams_p = cams[perm]
    tile_cams = tuple(
        tuple(dict.fromkeys(cams_p[128 * rt : 128 * (rt + 1)].tolist()))
        for rt in range(RT)
    )

    # per-core centers, cam-major with 48-col pad per group, pre-scaled,
    # transposed to [128, 2, PL] (partition=feature_lo, j=feature_hi)
    np_mm = NP_FP8 if MM == "fp8dr" else NP_BF16
    by_cam = centers.reshape(L, C, D)
    cenT_shards = []
    for k in range(NCORES):
        X = by_cam[k * L_LOCAL : (k + 1) * L_LOCAL]             # [1000, 8, 256]
        CP = np.zeros((GROUPS, GW, D), dtype=np.float32)
        for g in range(GROUPS):
            CP[g, 0:1000] = X[:, 2 * g, :]
            CP[g, 1000:2000] = X[:, 2 * g + 1, :]
        CP = (CEN_SCALE * CP).reshape(PL, 2, 128)
        cenT_shards.append(
            np.ascontiguousarray(CP.transpose(2, 1, 0), dtype=np_mm)
        )
    return centers, tile_cams, feats_p, labels_p, cams_p, cenT_shards


def kernel(feats, indexes, label_table, cam_table, centers):
    centers, tile_cams, feats_p, labels_p, cams_p, cenT_shards = _prepare(
        feats, indexes, label_table, cam_table, centers
    )
    nc = _build_program(tile_cams)
    runner = _get_runner(nc)
    runner.put_inputs(_make_in_maps(cenT_shards, feats_p))
    results = runner.execute()
    return _host_finish(results, feats_p, labels_p, cams_p, centers, tile_cams)


# revision 21
# speedup vs baseline: 1.8256x; 1.3050x over previous
"""Trainium2 Bass kernel for nn_CAPMemory (camera-aware proxy memory loss).

Strategy (8 NeuronCores, SPMD, no collectives):
  - Shard the 64000x256 proxy table over P: core k owns labels
    [1000k, 1000(k+1)), all 8 cameras. Per-core column layout is CAM-MAJOR,
    padded to 4 PSUM-bank groups of 2048 columns: group g = [cam 2g (1000
    cols) | cam 2g+1 (1000 cols) | 48 zero-pad cols]. Pad sims are 0 and can
    never reach the top-50; the intra exp never reads them.
  - Matmuls run in fp8(e4m3) DoubleRow mode: operands laid out [128, 2, free]
    so one matmul contracts all K=256 at 2 MACs/cell/cycle. Centers are
    pre-scaled by 16 on the host so their entries (~N(0,1/256)) sit in e4m3's
    normal range; the 1/16 rides in the host post-scale and the exp scale.
    feats are transposed/quantized on the host (fT input); row norms arrive
    as the sc20 input. Centers SBUF is double-buffered so the fp8 DMA of the
    next iteration hides under compute.
  - Each [128, 2048] f32 PSUM group is drained by per-chunk paths chosen
    statically (host+device share the plan) to balance ACT and DVE:
      exp    : ACT exp(sc20*sims) -> bf16 image + accum_out (the intra
               denominator, needed anyway). exp is monotone, so the image's
               top-8 are the slab's candidates (exp domain).
      copy   : ACT copies the chunk (1000 or merged 2048 cols) to bf16 SBUF.
      direct : DVE InstMax top-8 straight from PSUM (exact chunk top-8).
    bf16 images are folded by DVE pairwise tensor_max (2x bf16 mode) and
    finished with InstMax. InstMax/tensor_reduce have no 2x uops, so the
    fold+InstMax split is the cheapest DVE composition; copies route part of
    the scan to ACT, 'direct' keeps the rest on DVE.
  - Candidates: top-8 per chunk, 8 value-slots per (row-tile, group) pair
    = up to 512/row global. Folded chunks can miss a top-50 element that
    shares a fold stripe with a larger one; on this data that biases the
    final scalars ~1e-4 relative (gate 2e-2). Rows whose chunk 8th-largest
    exceeds the merged t50 are recomputed exactly on the host.
  - Host merge: intra logsumexp = log(sum_k srow_k); positives in f64;
    positive candidates removed per-chunk by value-matching against an
    fp8-simulated prediction of the device value; top-50 from the merged
    candidates; per-camera means as in the reference.
"""

import os
import sys
import functools

sys.path.insert(0, "/opt/trn_rl_repo")

import numpy as np

from concourse import bacc, mybir
from concourse.tile import TileContext

F32 = mybir.dt.float32
BF16 = mybir.dt.bfloat16
FP8 = mybir.dt.float8e4
NP_FP8 = mybir.dt.np(FP8)
NP_BF16 = mybir.dt.np(BF16)

N = 512          # batch
D = 256          # feature dim
L = 8000         # labels
C = 8            # cameras
NCORES = 8
L_LOCAL = 1000   # labels per core
RT = 4           # row tiles of 128
SLABW = 1024     # padded columns per camera slab (1000 + 24 pad)
PL = 8 * SLABW   # padded per-core columns (8192)
SW = 1000        # real slab width (one camera's columns)
INV_T = 20.0     # 1 / temperature
K = 50           # hard negatives
LW = 0.5         # inter-cam loss weight
CEN_SCALE = 16.0 # host pre-scale on centers (keeps fp8 in normal range)
CAND_PER_S = 8
SLABS = C
CAND = SLABS * CAND_PER_S    # 64 candidate values per row per core

# experiment knobs (defaults are the shipped config)
MM = os.environ.get("V2_MM", "fp8dr")            # fp8dr|bf16
N_COPY = int(os.environ.get("V2_COPY", "19"))    # no-exp slab-equivalents via ACT copy
FOLDS_TGT = int(os.environ.get("V2_FOLDS_TGT", "256"))  # fold images down to <= this
M1BUFS = int(os.environ.get("V2_M1BUFS", "3"))   # scr/fold tile ring depth


def _pair_order(sizes):
    """Order cameras big+small so most 128-row tiles span only 2 cameras."""
    desc = np.argsort(-np.asarray(sizes), kind="stable")
    big, small = desc[: C // 2], desc[C // 2 :][::-1]
    order = []
    for b, s in zip(big, small):
        order += [int(b), int(s)]
    return order


def _plan(tile_cams):
    """Chunk plan shared by device build and host decode.

    Returns plan[rt] = list of chunk descriptors (s = camera slab 0..7):
      ('direct', s, slot)                DVE InstMax straight from PSUM
      ('img', domain, [(s, kind)...], slot)
          1-2 writers ('exp' or 'copy') fill one bf16 image, which is folded
          on DVE and finished with one InstMax into candidate slot `slot`.
    domain is 'exp' or 'raw'; a slot covers all its writers' slabs. Of the
    no-exp slabs, ~N_COPY (spread evenly) go via the ACT-copy image path;
    same-domain images within a row tile are paired to share fold chains.
    """
    exp_w = [[s for s in range(SLABS) if s in tile_cams[rt]] for rt in range(RT)]
    noexp = [(rt, s) for rt in range(RT) for s in range(SLABS)
             if s not in tile_cams[rt]]
    n = len(noexp)
    ncopy = min(N_COPY, n)
    picked = set()
    for i in range(ncopy):
        picked.add(noexp[(i * n) // max(ncopy, 1)])

    out = []
    for rt in range(RT):
        chunks = []
        copy_w = []
        for rt2, s in noexp:
            if rt2 != rt:
                continue
            if (rt2, s) in picked:
                copy_w.append(s)
            else:
                chunks.append(("direct", s, s))
        for slabs, dom, kind in ((exp_w[rt], "exp", "exp"),
                                 (copy_w, "raw", "copy")):
            for i in range(0, len(slabs), 2):
                grp = [(s, kind) for s in slabs[i : i + 2]]
                chunks.append(("img", dom, grp, grp[0][0]))
        out.append(chunks)
    return out


@functools.lru_cache(maxsize=8)
def _build_program(tile_cams, repeats=1):
    nc = bacc.Bacc(None, target_bir_lowering=False, num_swdge_queues=4)

    mm_dt = FP8 if MM == "fp8dr" else BF16
    cenT = nc.dram_tensor("cenT", [128, 2, PL], mm_dt, kind="ExternalInput")
    fTd = nc.dram_tensor("fT", [RT, 128, 2, 128], mm_dt, kind="ExternalInput")
    sc20d = nc.dram_tensor("sc20", [128, RT], F32, kind="ExternalInput")
    candd = nc.dram_tensor("cand", [RT, 128, CAND], F32, kind="ExternalOutput")
    srowd = nc.dram_tensor("srow", [RT, 128, C], F32, kind="ExternalOutput")

    with TileContext(nc) as tc:
        with (
            tc.tile_pool(name="cen", bufs=2) as cenp,
            tc.tile_pool(name="ftp", bufs=2) as ftp,
            tc.tile_pool(name="m1p", bufs=M1BUFS) as m1p,
            tc.tile_pool(name="smallp", bufs=2) as smallp,
            tc.tile_pool(name="outp", bufs=2) as outp,
            tc.tile_pool(name="psum", bufs=4, space="PSUM") as psump,
        ):
            for _rep in range(repeats):
                _kernel_body(nc, tc, cenp, ftp, m1p, smallp, outp, psump,
                             cenT, fTd, sc20d, candd, srowd, tile_cams)

    nc.compile()
    return nc


def _fold_and_max(nc, m1p, co, img):
    """DVE: pairwise tensor_max folds (2x bf16 mode) down to <=FTGT stripe
    maxima, then InstMax top-8."""
    cur, w = img, img.shape[1]
    while w > FOLDS_TGT and w % 2 == 0:
        half = w // 2
        nxt = m1p.tile([128, half], BF16, name="fold")
        nc.vector.tensor_max(nxt[:, :], cur[:, 0:half], cur[:, half : 2 * half])
        cur, w = nxt, half
    nc.vector.max(co, cur[:, 0:w])


def _kernel_body(nc, tc, cenp, ftp, m1p, smallp, outp, psump,
                 cenT, fTd, sc20d, candd, srowd, tile_cams):
    ActF = mybir.ActivationFunctionType
    mm_dt = FP8 if MM == "fp8dr" else BF16
    plan = _plan(tile_cams)

    # small transfers first; warm the Exp LUT in ACT's idle window
    sc20_sb = smallp.tile([128, RT], F32, name="sc20", bufs=2)
    nc.sync.dma_start(out=sc20_sb[:, :], in_=sc20d[:, :])
    warm = smallp.tile([128, 1], F32, name="warm", bufs=2)
    nc.scalar.activation(warm[:, 0:1], sc20_sb[:, 0:1], ActF.Exp)

    fTs = []
    for rt in range(RT):
        fT = ftp.tile([128, 2, 128], mm_dt, name=f"fT{rt}")
        nc.scalar.dma_start(out=fT[:, :, :], in_=fTd[rt])
        fTs.append(fT)

    # centers: slab-major so early matmuls unblock first; spread issue
    # across engines/queues
    cen_sb = cenp.tile([128, 2, PL], mm_dt, name="cen")
    dma_engines = [nc.sync, nc.gpsimd]
    for g in range(4):
        s = slice(g * 2 * SLABW, (g + 1) * 2 * SLABW)
        for j in range(2):
            eng = dma_engines[(2 * g + j) % len(dma_engines)]
            eng.dma_start(out=cen_sb[:, j, s], in_=cenT[:, j, s])

    cand_ts = [
        outp.tile([128, CAND], F32, name=f"cand{rt}", bufs=2) for rt in range(RT)
    ]
    s_ts = [
        smallp.tile([128, C], F32, name=f"s_t{rt}", bufs=2) for rt in range(RT)
    ]

    for rt in range(RT):
        chunks = plan[rt]
        # slab -> (chunk index, writer position); image lazy alloc state
        slab_op = {}
        img_state = {}
        for ci, ch in enumerate(chunks):
            if ch[0] == "direct":
                slab_op[ch[1]] = (ci, 0)
            else:
                img_state[ci] = {"tile": None, "done": 0,
                                 "w": SW * len(ch[2])}
                for wi, (s, kind) in enumerate(ch[2]):
                    slab_op[s] = (ci, wi)

        def _cand_slot(slot):
            return cand_ts[rt][:, slot * CAND_PER_S : (slot + 1) * CAND_PER_S]

        for s in range(SLABS):
            ps = psump.tile([128, 2, 512], F32, name="ps")
            for mk in range(2):
                lo = s * SLABW + mk * 512
                if MM == "fp8dr":
                    nc.tensor.matmul(
                        ps[:, mk, :], fTs[rt][:, :, :],
                        cen_sb[:, :, lo : lo + 512],
                        start=True, stop=True,
                        perf_mode=mybir.MatmulPerfMode.DoubleRow,
                    )
                else:
                    nc.tensor.matmul(
                        ps[:, mk, :], fTs[rt][:, 0, :],
                        cen_sb[:, 0, lo : lo + 512],
                        start=True, stop=False,
                    )
                    nc.tensor.matmul(
                        ps[:, mk, :], fTs[rt][:, 1, :],
                        cen_sb[:, 1, lo : lo + 512],
                        start=False, stop=True,
                    )

            cols = ps.rearrange("p a b -> p (a b)")[:, 0:SW]
            ci, wi = slab_op[s]
            ch = chunks[ci]
            if ch[0] == "direct":
                nc.vector.max(_cand_slot(ch[2]), cols)
                continue
            _, dom, writers, slot = ch
            st = img_state[ci]
            if st["tile"] is None:
                st["tile"] = m1p.tile([128, st["w"]], BF16, name="img")
            dst = st["tile"][:, wi * SW : (wi + 1) * SW]
            kind = writers[wi][1]
            if kind == "exp":
                idx = tile_cams[rt].index(s)
                nc.scalar.activation(
                    dst, cols, ActF.Exp,
                    scale=sc20_sb[:, rt : rt + 1],
                    accum_out=s_ts[rt][:, idx : idx + 1],
                )
            else:
                nc.scalar.copy(dst, cols)
            st["done"] += 1
            if st["done"] == len(writers):
                _fold_and_max(nc, m1p, _cand_slot(slot), st["tile"])

        nc.sync.dma_start(out=candd[rt], in_=cand_ts[rt][:, :])
        nc.sync.dma_start(out=srowd[rt], in_=s_ts[rt][:, :])


class _Runner:
    """Sharded 8-core executor for a built Bass program (axon/PJRT path)."""

    def __init__(self, nc, n_cores=NCORES):
        import jax
        from jax.sharding import Mesh, PartitionSpec, NamedSharding
        from jax.experimental.shard_map import shard_map
        from concourse import bass2jax

        self.jax = jax
        self.nc = nc
        self.n_cores = n_cores
        bass2jax.install_neuronx_cc_hook()
        partition_name = (
            nc.partition_id_tensor.name if nc.partition_id_tensor else None
        )
        in_names, out_names, out_avals = [], [], []
        for alloc in nc.m.functions[0].allocations:
            if not isinstance(alloc, mybir.MemoryLocationSet):
                continue
            name = alloc.memorylocations[0].name
            if alloc.kind == "ExternalInput":
                if name != partition_name:
                    in_names.append(name)
            elif alloc.kind == "ExternalOutput":
                out_names.append(name)
                out_avals.append(
                    jax.core.ShapedArray(
                        tuple(alloc.tensor_shape), mybir.dt.np(alloc.dtype)
                    )
                )
        self.in_names, self.out_names, self.out_avals = in_names, out_names, out_avals
        n_params, n_outs = len(in_names), len(out_avals)
        all_in_names = list(in_names) + list(out_names)
        if partition_name is not None:
            all_in_names.append(partition_name)

        def _body(*args):
            operands = list(args)
            if partition_name is not None:
                operands.append(bass2jax.partition_id_tensor())
            return tuple(
                bass2jax._bass_exec_p.bind(
                    *operands,
                    out_avals=tuple(out_avals),
                    in_names=tuple(all_in_names),
                    out_names=tuple(out_names),
                    lowering_input_output_aliases=(),
                    sim_require_finite=True,
                    sim_require_nnan=True,
                    nc=nc,
                )
            )

        devices = jax.devices()[:n_cores]
        self.mesh = Mesh(np.asarray(devices), ("core",))
        self.sh = NamedSharding(self.mesh, PartitionSpec("core"))
        self.fn = jax.jit(
            shard_map(
                _body,
                mesh=self.mesh,
                in_specs=(PartitionSpec("core"),) * (n_params + n_outs),
                out_specs=(PartitionSpec("core"),) * n_outs,
                check_rep=False,
            ),
            donate_argnums=tuple(range(n_params, n_params + n_outs)),
            keep_unused=True,
        )
        self._zero_shapes = [
            ((n_cores * a.shape[0], *a.shape[1:]), a.dtype) for a in out_avals
        ]

    def put_inputs(self, in_maps):
        self.dev_in = [
            self.jax.device_put(
                np.concatenate([np.asarray(m[name]) for m in in_maps], axis=0),
                self.sh,
            )
            for name in self.in_names
        ]

    def _zeros(self):
        return [
            self.jax.device_put(np.zeros(s, d), self.sh)
            for s, d in self._zero_shapes
        ]

    def execute(self):
        outs = self.fn(*self.dev_in, *self._zeros())
        self.jax.block_until_ready(outs)
        return self.unpack(outs)

    def unpack(self, outs):
        return [
            {
                name: np.asarray(outs[i]).reshape(
                    self.n_cores, *self.out_avals[i].shape
                )[c]
                for i, name in enumerate(self.out_names)
            }
            for c in range(self.n_cores)
        ]


_RUNNERS = {}
_LAST_FALLBACKS = 0
_FORCE_FALLBACK = False  # test hook: exercise the exact host fallback path


def _get_runner(nc):
    r = _RUNNERS.get(id(nc))
    if r is None:
        r = _Runner(nc)
        _RUNNERS[id(nc)] = r
    return r


def _make_in_maps(cenT_shards, feats_p):
    np_mm = NP_FP8 if MM == "fp8dr" else NP_BF16
    inv = 1.0 / np.linalg.norm(feats_p.astype(np.float64), axis=1)
    sc20 = np.ascontiguousarray(
        (INV_T / CEN_SCALE) * inv.reshape(RT, 128).T, dtype=np.float32
    )  # [128, RT]
    # fT[rt, p, j, m] = feats_p[rt*128 + m, 128*j + p]
    fT = np.ascontiguousarray(
        feats_p.reshape(RT, 128, 2, 128).transpose(0, 3, 2, 1), dtype=np_mm
    )
    return [
        {"cenT": cenT_shards[k], "fT": fT, "sc20": sc20}
        for k in range(NCORES)
    ]


def _host_finish(results, feats_p, labels_p, cams_p, centers, tile_cams):
    rows = np.arange(N)
    invn = 1.0 / np.linalg.norm(feats_p.astype(np.float64), axis=1)
    plan = _plan(tile_cams)
    # chunk tables: slab (camera) -> covering chunk slot + kind, per rt
    slab_slot = np.full((RT, SLABS), -1, dtype=np.int64)
    slab_kind = [[None] * SLABS for _ in range(RT)]
    active = np.zeros((RT, SLABS), dtype=bool)     # slots that carry values
    exp_slot = np.zeros((RT, SLABS), dtype=bool)   # slot domain is exp
    for rt in range(RT):
        for ch in plan[rt]:
            if ch[0] == "direct":
                _, s, slot = ch
                writers = [(s, "direct")]
                dom = "raw"
            else:
                _, dom, writers, slot = ch
            active[rt, slot] = True
            exp_slot[rt, slot] = dom == "exp"
            for s, kind in writers:
                slab_slot[rt, s] = slot
                slab_kind[rt][s] = kind

    cand_raw = np.stack(
        [results[k]["cand"].reshape(N, SLABS, CAND_PER_S) for k in range(NCORES)]
    ).astype(np.float64)  # [8, 512, 8slots, 8]
    cscale = invn / CEN_SCALE
    rt_of = rows // 128
    is_exp = exp_slot[rt_of]                       # [512, 8slots]
    act = active[rt_of]                            # [512, 8slots]
    cand = np.where(
        is_exp[None, :, :, None],
        np.log(np.maximum(cand_raw, 1e-30)) / INV_T,
        cand_raw * cscale[None, :, None, None],
    )
    cand = np.where(act[None, :, :, None], cand, -np.inf)

    # srow slots: per row-tile, slot idx corresponds to tile_cams order
    slot = np.zeros(N, dtype=np.int64)
    for rt in range(RT):
        for idx, cam in enumerate(tile_cams[rt]):
            sel = slice(128 * rt, 128 * (rt + 1))
            slot[sel] = np.where(cams_p[sel] == cam, idx, slot[sel])
    p_of = rows % 128
    s_k = np.stack(
        [
            results[k]["srow"].reshape(RT, 128, C)[rt_of, p_of, slot]
            for k in range(NCORES)
        ]
    ).astype(np.float64)  # [8, 512]

    fe = feats_p.astype(np.float64)
    fn = fe / np.linalg.norm(fe, axis=1, keepdims=True)
    cen = centers.astype(np.float64)

    # positives: 8 same-label proxies per row (host, f64)
    gidx = labels_p[:, None] * C + np.arange(C)[None, :]        # [512, 8]
    pos = np.einsum("rcd,rd->rc", cen[gidx], fn)                # [512, 8]

    # ---- intra ----
    lse_intra = np.log(s_k.sum(axis=0))
    v = pos[np.arange(N), cams_p]
    loss_intra_i = lse_intra - INV_T * v

    # ---- inter: remove positives from candidates by value, then top-50 ----
    np_mm = NP_FP8 if MM == "fp8dr" else NP_BF16
    f_q = feats_p.astype(np_mm).astype(np.float64)
    g_q = (CEN_SCALE * centers[gidx]).astype(np_mm).astype(np.float64)
    pos_dev = np.einsum("rcd,rd->rc", g_q, f_q).astype(np.float32)  # raw dot
    sc20r = (INV_T / CEN_SCALE) * invn
    pred_exp = (
        np.log(
            np.exp(sc20r[:, None] * pos_dev.astype(np.float64))
            .astype(NP_BF16).astype(np.float64)
        ) / INV_T
    )
    pred_raw_b = pos_dev.astype(NP_BF16).astype(np.float64) * cscale[:, None]
    pred_raw_x = pos_dev.astype(np.float64) * cscale[:, None]

    CRS = cand.transpose(1, 0, 2, 3)                       # [512, 8cores, 8, 8]
    owner = labels_p // L_LOCAL
    for i in rows:
        rt = i // 128
        for c in range(C):
            kind = slab_kind[rt][c]
            sl = slab_slot[rt, c]
            if kind == "exp":
                pv = pred_exp[i, c]
            elif kind == "copy":
                pv = pred_raw_b[i, c]
            else:
                pv = pred_raw_x[i, c]
            vals = CRS[i, owner[i], sl]
            d = np.abs(vals - pv)
            j = np.argmin(d)
            if d[j] < 2.5e-4 + 5e-3 * abs(pv):
                CRS[i, owner[i], sl, j] = -np.inf

    CR = CRS.reshape(N, NCORES * CAND)
    part = np.partition(CR, NCORES * CAND - K, axis=1)[:, -K:]  # top-50 values
    t50 = part.min(axis=1)

    # at-risk check: each chunk's 8th-largest candidate should be <= t50
    # (sound certificate for 'direct' chunks, heuristic for folded chunks)
    slab8 = np.where(act[None], cand[:, :, :, CAND_PER_S - 1], -np.inf)
    if _FORCE_FALLBACK:
        bad = rows
    else:
        bad = np.where(slab8.max(axis=(0, 2)) > t50)[0]
    global _LAST_FALLBACKS
    _LAST_FALLBACKS = len(bad)
    for i in bad:
        sims_row = cen @ fn[i]                                  # [64000] exact
        sims_row[C * labels_p[i] : C * labels_p[i] + C] = -np.inf
        part[i] = np.sort(sims_row)[-K:]

    z = np.concatenate([pos, part], axis=1) * INV_T             # [512, 58]
    mz = z.max(axis=1)
    lse_inter = np.log(np.exp(z - mz[:, None]).sum(axis=1)) + mz
    loss_inter_i = lse_inter - INV_T * pos.mean(axis=1)

    # ---- per-camera means, summed ----
    cnt = np.bincount(cams_p, minlength=C).astype(np.float64)
    s_intra = np.bincount(cams_p, weights=loss_intra_i, minlength=C)
    s_inter = np.bincount(cams_p, weights=loss_inter_i, minlength=C)
    safe = np.maximum(cnt, 1.0)
    li = np.sum(np.where(cnt > 0, s_intra / safe, 0.0))
    le = LW * np.sum(np.where(cnt > 0, s_inter / safe, 0.0))
    return np.array([li, le], dtype=np.float32)


def _prepare(feats, indexes, label_table, cam_table, centers):
    feats = np.asarray(feats, dtype=np.float32)
    indexes = np.asarray(indexes)
    label_table = np.asarray(label_table)
    cam_table = np.asarray(cam_table)
    centers = np.asarray(centers, dtype=np.float32)

    labels = np.asarray(label_table[indexes], dtype=np.int64)
    cams = np.asarray(cam_table[indexes], dtype=np.int64)

    # permute rows so camera groups are contiguous, ordered big+small so most
    # 128-row tiles span only ~2 cameras (fewer intra exp instructions)
    sizes = np.bincount(cams, minlength=C)
    order = _pair_order(sizes)
    perm = np.concatenate([np.where(cams == c)[0] for c in order])
    feats_p = np.ascontiguousarray(feats[perm])
    labels_p = labels[perm]
    cams_p = cams[perm]
    tile_cams = tuple(
        tuple(dict.fromkeys(cams_p[128 * rt : 128 * (rt + 1)].tolist()))
        for rt in range(RT)
    )

    # per-core centers, cam-major with 48-col pad per group, pre-scaled,
    # transposed to [128, 2, PL] (partition=feature_lo, j=feature_hi)
    np_mm = NP_FP8 if MM == "fp8dr" else NP_BF16
    by_cam = centers.reshape(L, C, D)
    cenT_shards = []
    for k in range(NCORES):
        X = by_cam[k * L_LOCAL : (k + 1) * L_LOCAL]             # [1000, 8, 256]
        CP = np.zeros((C, SLABW, D), dtype=np.float32)
        for c in range(C):
            CP[c, 0:SW] = X[:, c, :]
        CP = (CEN_SCALE * CP).reshape(PL, 2, 128)
        cenT_shards.append(
            np.ascontiguousarray(CP.transpose(2, 1, 0), dtype=np_mm)
        )
    return centers, tile_cams, feats_p, labels_p, cams_p, cenT_shards


def kernel(feats, indexes, label_table, cam_table, centers):
    centers, tile_cams, feats_p, labels_p, cams_p, cenT_shards = _prepare(
        feats, indexes, label_table, cam_table, centers
    )
    nc = _build_program(tile_cams)
    runner = _get_runner(nc)
    runner.put_inputs(_make_in_maps(cenT_shards, feats_p))
    results = runner.execute()
    return _host_finish(results, feats_p, labels_p, cams_p, centers, tile_cams)
